# revision 34
# baseline (speedup 1.0000x reference)
"""Trainium2 Bass kernel for nn_MetaBlock (dense transformer flow block).

Self-contained: takes FULL inputs, shards batch across 8 NeuronCores
(cores 0-3 compute batch 0, cores 4-7 batch 1 — SPMD same program),
returns full outputs (x_out [B,L,IN], logdet [B]).

Key algebraic restructurings (exact, validated vs reference):
  * The [B,L,L,C] pair tensor collapses: einsum('bijc,ch->bhij', pair, ph)
    == dist[b,i,j]*alpha[l,h] + beta[l,h] with alpha = pair_w @ ph[l,:,h];
    beta is constant per softmax row -> drops out of softmax entirely.
  * Softmax without max-subtraction (logits are O(1) by construction);
    causal mask via live-extent trimming + lower-tri multiply on the
    diagonal 128-block of e.
  * Attention fully "transposed" (logits^T[j,i]): the exp IS the PSUM
    drain, no attention-matrix transposes; alpha*dist added in-PSUM via
    a scaled-identity matmul; Z arrives free via an appended ones-column
    on V; 1/Z computed in a transposed [128, 4] layout (cheap DVE
    reciprocal) and broadcast back per-head via small DMA reshapes.
"""

import sys

sys.path.insert(0, "/opt/trn_rl_repo")

import numpy as np
import ml_dtypes

import concourse.bass as bass
import concourse.bacc as bacc
import concourse.mybir as mybir
import concourse.tile as tile
from concourse.bass_utils import run_bass_kernel_spmd
from concourse.masks import make_identity
import concourse.hw_specs as hw_specs
import concourse.bacc as _bacc_for_patch  # noqa

# Pin ACT table sets: keep set IDs stable but hide every set except the two
# we use, so the chooser never alternates between overlapping sets (each
# swap costs ~1.3us of ACT_TABLE_LOAD).
_ACT_KEEP = ("natural_log_exp_and_others", "gelu_apprx_tanh_and_others")
_orig_get_tables = hw_specs.get_activation_tables


def _pinned_tables(arch):
    t = _orig_get_tables(arch)
    return {k: (v if k in _ACT_KEEP else set()) for k, v in t.items()}


hw_specs.get_activation_tables = _pinned_tables
_bacc_for_patch.get_activation_tables = _pinned_tables

F32 = mybir.dt.float32
F32R = mybir.dt.float32r
BF16 = mybir.dt.bfloat16
AF = mybir.ActivationFunctionType
ALU = mybir.AluOpType

B, L, C, H, HD, NL, IN, EXP = 2, 512, 256, 4, 64, 2, 3, 4
N1 = EXP * C          # 1024
IC = L // 128         # 4 i-chunks
KT = C // 128         # 2 C k-tiles
NT = N1 // 128        # 8 mlp-hidden chunks
INV_HD = 1.0 / np.sqrt(HD)
EPS = 1e-5


def _r(x):
    return x.bitcast(F32R)


def build_program(flags):
    """Trace the full single-core program. flags: dict of bools enabling the
    optional bias/scale paths (emitted only when the input is nonzero)."""
    nc = bacc.Bacc("TRN2", target_bir_lowering=False, debug=False,
                   num_swdge_queues=4)

    dt_ = nc.dram_tensor
    # mega-packed inputs: per-layer bf16 weight pack, shared bf16 pack,
    # f32 pack (few large DMAs instead of ~90 small ones)
    CB = 7168   # Wqk 1024 | Wv 512 | W1 2048 | W2 2048 | aI 512 | Wo 1024(rows 0:64)
    CP = 3472   # condT 1024 | xT 512 | xT2 512 | sq 512 | tri 128 | Wout 12 | Win 256 | Wc 512 (+pad)
    CF = 1554   # posb 1024 | b1 16 | poba 1 | pobb 1 | xTf 512
    d_in = {
        "packB": dt_("packB", [NL, 128, CB], BF16, kind="ExternalInput"),
        "pinB": dt_("pinB", [128, CP], BF16, kind="ExternalInput"),
        "packF": dt_("packF", [128, CF], F32, kind="ExternalInput"),
    }
    xout_d = dt_("xout", [IN, L], F32, kind="ExternalOutput")
    ld_d = dt_("logdet", [1, 1], F32, kind="ExternalOutput")

    with tile.TileContext(nc) as tc, \
         tc.tile_pool(name="per", bufs=1) as per, \
         tc.tile_pool(name="wk", bufs=2) as wk, \
         tc.tile_pool(name="wk3", bufs=3) as wk3, \
         tc.tile_pool(name="ps", bufs=2, space="PSUM") as ps, \
         tc.tile_pool(name="psB", bufs=4, space="PSUM") as psB, \
         tc.tile_pool(name="dr", bufs=4, space="DRAM") as dr, \
         tc.tile_pool(name="wk4", bufs=4) as wk4:

        dma = nc.sync.dma_start

        # ---------------- persistent SBUF state ----------------
        # issue loads in first-use order: prologue inputs, then layer packs
        PIN_s = per.tile([128, CP], BF16, tag="PIN", name="PIN")
        dma(out=PIN_s[:, :], in_=d_in["pinB"].ap())
        PF_s = per.tile([128, CF], F32, tag="PF", name="PF")
        dma(out=PF_s[:, :], in_=d_in["packF"].ap())
        PB_s = [per.tile([128, CB], BF16, tag=f"PB{l}", name=f"PB{l}")
                for l in range(NL)]
        for l in range(NL):
            dma(out=PB_s[l][:, :], in_=d_in["packB"].ap()[l])

        def pb(l, off, w):
            return PB_s[l][:, off:off + w]
        Wqk_s = [[pb(l, k * 512, 512) for k in range(KT)] for l in range(NL)]
        Wv_s = [[pb(l, 1024 + k * 256, 256) for k in range(KT)] for l in range(NL)]
        W1_s = [[pb(l, 1536 + k * N1, N1) for k in range(KT)] for l in range(NL)]
        W2_s = [[pb(l, 3584 + n * 256, 256) for n in range(NT)] for l in range(NL)]
        aI_sl = [[pb(l, 5632 + hh * 128, 128) for hh in range(H)] for l in range(NL)]
        Wo_s = [[PB_s[l][0:HD, 6144 + hh * 256:6144 + (hh + 1) * 256]
                 for hh in range(H)] for l in range(NL)]
        condT_s = [PIN_s[:, k * 512:(k + 1) * 512] for k in range(KT)]
        xT_s = PIN_s[0:IN, 1024:1024 + L]
        xT2_s = PIN_s[0:IN, 1536:1536 + L]
        sq_s = PIN_s[0:1, 2048:2048 + L]
        tri_s = PIN_s[:, 2560:2560 + 128]
        Wout_s = [PIN_s[:, 2688 + k * 2 * IN:2688 + (k + 1) * 2 * IN]
                  for k in range(KT)]
        Win_s = PIN_s[0:IN, 2700:2700 + C]
        Wc_s = [PIN_s[:, 2956 + k * C:2956 + (k + 1) * C] for k in range(KT)]
        posb_s = [PF_s[:, i * C:(i + 1) * C] for i in range(IC)]
        b1_s = [PF_s[:, 1024 + l * NT:1024 + (l + 1) * NT] for l in range(NL)]
        poba_s = PF_s[0:IN, 1040:1041]
        pobb_s = PF_s[0:IN, 1041:1042]
        xTf_s = PF_s[0:IN, 1042:1042 + L]

        opt_s = {}
        if flags["bqk"]:
            opt_s["bqk"] = [load(f"bqk{l}", [128, 2 * KT], F32,
                                 d_in["bqk"].ap()[l]) for l in range(NL)]
        for nm in ("bvb", "obb", "b2b", "ln1s", "ln1b", "ln2s", "ln2b"):
            if flags[nm]:
                opt_s[nm] = [load(f"{nm}{l}", [128, C], F32, d_in[nm].ap()[l])
                             for l in range(NL)]
        if flags["bcb"]:
            opt_s["bcb"] = load("bcb_s", [128, C], F32, d_in["bcb"].ap())

        ident = per.tile([128, 128], BF16, tag="ident", name="ident")
        make_identity(nc, ident[:, :])
        identf = per.tile([128, 128], F32, tag="identf", name="identf")
        make_identity(nc, identf[:, :])
        eps_t = per.tile([128, 1], F32, tag="eps", name="eps_t")
        nc.vector.memset(eps_t[:, :], EPS)
        ones_r = per.tile([1, L], BF16, tag="ones_r", name="ones_r")
        nc.vector.memset(ones_r[:, :], 1.0)
        ones3 = per.tile([IN, 1], F32, tag="ones3", name="ones3")
        nc.vector.memset(ones3[:, :], 1.0)

        h_s = [per.tile([128, C], F32, tag=f"h{i}", name=f"h{i}") for i in range(IC)]
        ce_s = [per.tile([128, C], F32, tag=f"ce{i}", name=f"ce{i}") for i in range(IC)]
        dist_s = [per.tile([128, L], BF16, tag=f"dist{j}", name=f"dist{j}")
                  for j in range(IC)]
        vA_s = [per.tile([128, H * (HD + 1)], BF16, tag=f"vA{j}", name=f"vA{j}")
                for j in range(IC)]
        for j in range(IC):
            for hh in range(H):
                nc.gpsimd.memset(vA_s[j][:, hh * 65 + 64: hh * 65 + 65], 1.0)


        # touch DMA-loaded operands once per engine so later consumers carry
        # at most one cross-engine wait (ISA sync-slot limit per instruction)
        tchv = per.tile([128, 1], F32, tag="tchv", name="tchv")
        tcha = per.tile([128, 1], F32, tag="tcha", name="tcha")
        nc.vector.tensor_copy(out=tchv[:, :], in_=posb_s[0][:, 0:1])
        nc.vector.tensor_copy(out=tchv[0:IN, :], in_=xTf_s[:, 0:1])
        nc.vector.tensor_copy(out=tchv[:, :].bitcast(BF16)[:, 0:1],
                              in_=tri_s[:, 0:1])
        nc.scalar.copy(tcha[:, :], b1_s[0][:, 0:1])
        nc.scalar.copy(tcha[0:IN, :], poba_s)
        nc.scalar.copy(tcha[0:IN, :], pobb_s)

        mm = nc.tensor.matmul

        def transpose128(pout, src):
            nc.tensor.transpose(pout, src, ident[:, :])

        def transpose128f(pout, src):
            nc.tensor.transpose(pout, src, identf[:, :])

        def drain(dst, src, use_act):
            if use_act:
                nc.scalar.copy(dst, src)
            else:
                nc.vector.tensor_copy(out=dst, in_=src)

        # ---------------- prologue: dist, h0, cond_emb ----------------
        for j in range(IC):
            pd = psB.tile([128, L], F32, tag="big", name=f"pd{j}")
            sl = slice(j * 128, (j + 1) * 128)
            mm(pd[:, :], xT2_s[:, sl], xT_s[:, :], start=True, stop=False)
            mm(pd[:, :], sq_s[:, sl], ones_r[:, :], start=False, stop=False)
            mm(pd[:, :], ones_r[:, sl], sq_s[:, :], start=False, stop=True)
            nc.vector.tensor_scalar_max(out=pd[:, :], in0=pd[:, :], scalar1=EPS)
            lnd = wk.tile([128, L], F32, tag="lnd", name="lnd")
            nc.scalar.activation(lnd[:, :], pd[:, :], AF.Ln,
                                 bias=eps_t[:, :], scale=1.0)
            nc.scalar.activation(dist_s[j][:, :], lnd[:, :], AF.Exp, scale=0.5)
        for i in range(IC):
            sl = slice(i * 128, (i + 1) * 128)
            ph = ps.tile([128, C], F32, tag="small", name=f"ph{i}")
            mm(ph[:, :], xT_s[:, sl], Win_s[:, :], start=True, stop=True)
            nc.vector.tensor_add(h_s[i][:, :], ph[:, :], posb_s[i][:, :])
            pc = ps.tile([128, C], F32, tag="small", name=f"pc{i}")
            for k in range(KT):
                mm(pc[:, :], condT_s[k][:, sl], Wc_s[k][:, :],
                     start=(k == 0), stop=(k == KT - 1))
            if flags["bcb"]:
                nc.vector.tensor_add(ce_s[i][:, :], pc[:, :], opt_s["bcb"][:, :])
            else:
                nc.vector.tensor_copy(out=ce_s[i][:, :], in_=pc[:, :])

        # ---------------- layers ----------------
        for l in range(NL):
            for i in range(IC):
                nc.vector.tensor_add(h_s[i][:, :], h_s[i][:, :], ce_s[i][:, :])

            def layernorm(sname, bname, out_tiles):
                for i in range(IC):
                    st = wk.tile([128, 6], F32, tag="bnst", name="st")
                    nc.vector.bn_stats(out=st[:, :], in_=h_s[i][:, :])
                    mv = wk.tile([128, 2], F32, tag="bnmv", name="mv")
                    nc.vector.bn_aggr(out=mv[:, :], in_=st[:, :])
                    sd = wk.tile([128, 1], F32, tag="sd", name="sd")
                    nc.scalar.activation(sd[:, :], mv[:, 1:2], AF.Ln,
                                         bias=eps_t[:, :], scale=1.0)
                    rs = wk.tile([128, 1], F32, tag="rs", name="rs")
                    nc.scalar.activation(rs[:, :], sd[:, :], AF.Exp, scale=-0.5)
                    at = out_tiles[i]
                    nc.vector.tensor_scalar(
                        out=at[:, :], in0=h_s[i][:, :], scalar1=mv[:, 0:1],
                        scalar2=rs[:, :], op0=ALU.subtract, op1=ALU.mult)
                    if flags[sname]:
                        nc.vector.tensor_mul(at[:, :], at[:, :], opt_s[sname][l][:, :])
                    if flags[bname]:
                        nc.vector.tensor_add(at[:, :], at[:, :], opt_s[bname][l][:, :])

            a_t = [wk3.tile([128, C], BF16, tag=f"a{i}", name=f"a{i}") for i in range(IC)]
            layernorm("ln1s", "ln1b", a_t)

            aT_t = [wk3.tile([128, L], BF16, tag=f"aT{k}", name=f"aT{k}")
                    for k in range(KT)]
            for k in range(KT):
                for i in range(IC):
                    pt = ps.tile([128, 128], BF16, tag="small", name="pt")
                    transpose128(pt[:, :], a_t[i][:, k * 128:(k + 1) * 128])
                    drain(aT_t[k][:, i * 128:(i + 1) * 128], pt[:, :], k % 2)

            qk_t = [wk3.tile([128, L], BF16, tag=f"qk{m}", name=f"qk{m}")
                    for m in range(2 * KT)]
            for m in range(2 * KT):
                pq = psB.tile([128, L], F32, tag="big", name="pq")
                for k in range(KT):
                    mm(pq[:, :], Wqk_s[l][k][:, m * 128:(m + 1) * 128],
                         aT_t[k][:, :], start=(k == 0), stop=(k == KT - 1))
                if flags["bqk"]:
                    nc.vector.tensor_scalar(
                        out=qk_t[m][:, :], in0=pq[:, :],
                        scalar1=opt_s["bqk"][l][:, m:m + 1], scalar2=None,
                        op0=ALU.add)
                else:
                    drain(qk_t[m][:, :], pq[:, :], m % 2)

            for j in range(IC):
                sl = slice(j * 128, (j + 1) * 128)
                pv = ps.tile([128, C], F32, tag="small", name="pv")
                for k in range(KT):
                    mm(pv[:, :], aT_t[k][:, sl], Wv_s[l][k][:, :],
                         start=(k == 0), stop=(k == KT - 1))
                if flags["bvb"]:
                    nc.vector.tensor_add(pv[:, :], pv[:, :], opt_s["bvb"][l][:, :])
                nc.vector.tensor_copy(
                    out=vA_s[j].rearrange("p (h x) -> p h x", h=H)[:, :, 0:HD],
                    in_=pv.rearrange("p (h x) -> p h x", h=H)[:, :, :])

            oT_t = {}
            eT_all = {}
            # phase 1: all heads' logits + exp (dense PE stream keeps HAM warm)
            for hh in range(H):
                qt = qk_t[hh // 2]
                kt_ = qk_t[KT + hh // 2]
                rq = slice((hh % 2) * HD, (hh % 2) * HD + HD)
                eT = [wk.tile([128, L], BF16, tag=f"e{hh}_{j}", name=f"e{hh}_{j}")
                      for j in range(IC)]
                for j in range(IC):
                    lo = j * 128
                    pl = psB.tile([128, L], F32, tag="big", name="pl")
                    mm(pl[:, lo:], kt_[rq, lo:lo + 128], qt[rq, lo:],
                         start=True, stop=False)
                    mm(pl[:, lo:], aI_sl[l][hh],
                       dist_s[j][:, lo:], start=False, stop=True)
                    nc.scalar.activation(eT[j][:, lo:], pl[:, lo:], AF.Exp)
                    nc.vector.tensor_mul(eT[j][:, lo:lo + 128],
                                         eT[j][:, lo:lo + 128], tri_s[:, :])
                eT_all[hh] = eT
            # phase 2: all heads' o-accumulation + 1/Z normalize
            for hh in range(H):
                eT = eT_all[hh]
                po = ps.tile([HD + 1, L], F32, tag="po", name="po")
                for j in range(IC):
                    lo = j * 128
                    mm(po[:, lo:], vA_s[j][:, hh * 65:(hh + 1) * 65],
                       eT[j][:, lo:], start=(j == 0), stop=(j == IC - 1))
                zr = wk4.tile([1, L], F32, tag="zrow", name="zr")
                nc.scalar.activation(zr[:, :], po[HD:HD + 1, :], AF.Ln)
                rr = wk4.tile([1, L], F32, tag="rrow", name="rr")
                nc.scalar.activation(rr[:, :], zr[:, :], AF.Exp, scale=-1.0)
                rb = wk4.tile([HD, L], F32, tag="rb", name="rb")
                nc.gpsimd.partition_broadcast(rb[:, :], rr[0:1, :])
                ot = wk.tile([HD, L], BF16, tag=f"oT{hh}", name=f"ot{hh}")
                nc.vector.tensor_mul(ot[:, :], po[0:HD, :], rb[:, :])
                oT_t[hh] = ot

            for i in range(IC):
                sl = slice(i * 128, (i + 1) * 128)
                pp = ps.tile([128, C], F32, tag="small", name="pp")
                for hh in range(H):
                    mm(pp[:, :], oT_t[hh][:, sl], Wo_s[l][hh][:, :],
                       start=(hh == 0), stop=(hh == H - 1))
                nc.vector.tensor_add(h_s[i][:, :], pp[:, :], h_s[i][:, :])
                if flags["obb"]:
                    nc.vector.tensor_add(h_s[i][:, :], h_s[i][:, :],
                                         opt_s["obb"][l][:, :])

            m_t = [wk3.tile([128, C], BF16, tag=f"a{i}", name=f"m{i}")
                   for i in range(IC)]
            layernorm("ln2s", "ln2b", m_t)
            mT_t = [wk3.tile([128, L], BF16, tag=f"mT{k}", name=f"mT{k}")
                    for k in range(KT)]
            for k in range(KT):
                for i in range(IC):
                    pt = ps.tile([128, 128], BF16, tag="small", name="pt")
                    transpose128(pt[:, :], m_t[i][:, k * 128:(k + 1) * 128])
                    drain(mT_t[k][:, i * 128:(i + 1) * 128], pt[:, :], k % 2)
            gT_t = [wk.tile([128, L], BF16, tag=f"gT{n}", name=f"gT{n}")
                    for n in range(NT)]
            for n in range(NT):
                pz = psB.tile([128, L], F32, tag="big", name="pz")
                for k in range(KT):
                    mm(pz[:, :], W1_s[l][k][:, n * 128:(n + 1) * 128],
                         mT_t[k][:, :], start=(k == 0), stop=(k == KT - 1))
                nc.scalar.activation(gT_t[n][:, :], pz[:, :], AF.Gelu_apprx_tanh,
                                     bias=b1_s[l][:, n:n + 1], scale=1.0)
            for i in range(IC):
                sl = slice(i * 128, (i + 1) * 128)
                pz2 = ps.tile([128, C], F32, tag="small", name="pz2")
                for n in range(NT):
                    mm(pz2[:, :], gT_t[n][:, sl], W2_s[l][n][:, :],
                       start=(n == 0), stop=(n == NT - 1))
                nc.vector.tensor_add(h_s[i][:, :], pz2[:, :], h_s[i][:, :])
                if flags["b2b"]:
                    nc.vector.tensor_add(h_s[i][:, :], h_s[i][:, :],
                                         opt_s["b2b"][l][:, :])

        # ---------------- epilogue ----------------
        hT_t = [wk3.tile([128, L], BF16, tag=f"aT{k}", name=f"hT{k}")
                for k in range(KT)]
        for k in range(KT):
            for i in range(IC):
                ptf = ps.tile([128, 128], F32, tag="small", name="ptf")
                transpose128f(ptf[:, :], h_s[i][:, k * 128:(k + 1) * 128])
                drain(hT_t[k][:, i * 128:(i + 1) * 128], ptf[:, :], k % 2)
        pya = ps.tile([IN, L], F32, tag="po", name="pya")
        pyb = ps.tile([IN, L], F32, tag="po", name="pyb")
        for k in range(KT):
            mm(pya[:, :], Wout_s[k][:, 0:IN], hT_t[k][:, :],
                 start=(k == 0), stop=(k == KT - 1))
            mm(pyb[:, :], Wout_s[k][:, IN:2 * IN], hT_t[k][:, :],
                 start=(k == 0), stop=(k == KT - 1))
        ya = wk.tile([IN, L], F32, tag="ya", name="ya")
        nc.scalar.activation(ya[:, :], pya[:, :], AF.Identity,
                             bias=poba_s, scale=1.0)
        yb = wk.tile([IN, L], F32, tag="yb", name="yb")
        nc.scalar.activation(yb[:, :], pyb[:, :], AF.Identity,
                             bias=pobb_s, scale=1.0)
        # causal shift: xa[:, i] = ya[:, i-1], col 0 = 0
        xa = wk.tile([IN, L], F32, tag="xa", name="xa")
        nc.vector.memset(xa[:, 0:1], 0.0)
        nc.vector.tensor_copy(out=xa[:, 1:L], in_=ya[:, 0:L - 1])
        xb = wk.tile([IN, L], F32, tag="xb", name="xb")
        nc.vector.memset(xb[:, 0:1], 0.0)
        nc.vector.tensor_copy(out=xb[:, 1:L], in_=yb[:, 0:L - 1])
        ea = wk.tile([IN, L], F32, tag="ea", name="ea")
        nc.scalar.activation(ea[:, :], xa[:, :], AF.Exp, scale=-1.0)
        xd = wk.tile([IN, L], F32, tag="xd", name="xd")
        nc.vector.tensor_tensor(out=xd[:, :], in0=xTf_s[:, :],
                                in1=xb[:, :], op=ALU.subtract)
        xo = wk.tile([IN, L], F32, tag="xo", name="xo")
        nc.vector.tensor_mul(xo[:, :], xd[:, :], ea[:, :])
        dma(out=xout_d.ap(), in_=xo[:, :])
        sa = wk.tile([IN, 1], F32, tag="sa", name="sa")
        nc.vector.reduce_sum(out=sa[:, :], in_=xa[:, :],
                             axis=mybir.AxisListType.X)
        pld = ps.tile([1, 1], F32, tag="po", name="pld")
        mm(pld[:, :], sa[:, :], ones3[:, :], start=True, stop=True)
        ldt = wk.tile([1, 1], F32, tag="ldt", name="ldt")
        nc.scalar.mul(ldt[:, :], pld[:, :], -1.0 / (L * IN))
        dma(out=ld_d.ap(), in_=ldt[:, :])

    nc.compile()
    return nc


def host_prep(inputs):
    """Host-side sharding/layout prep only (transposes, reshapes, tiny
    pair_w @ pairhead_w contraction, mega-packing into few DMA tensors)."""
    bf = ml_dtypes.bfloat16
    g = {k: np.asarray(v, np.float32) for k, v in inputs.items()}
    flags = {
        "bqk": bool(np.any(g["qkv_b"][:, :2 * C])),
        "bvb": bool(np.any(g["qkv_b"][:, 2 * C:])),
        "bcb": bool(np.any(g["proj_cond_b"])),
        "obb": bool(np.any(g["out_b"])),
        "b2b": bool(np.any(g["mlp2_b"])),
        "ln1s": bool(np.any(g["ln1_s"] != 1.0)),
        "ln1b": bool(np.any(g["ln1_b"])),
        "ln2s": bool(np.any(g["ln2_s"] != 1.0)),
        "ln2b": bool(np.any(g["ln2_b"])),
    }
    if any(flags.values()):
        raise NotImplementedError(f"nonzero optional bias/scale: {flags}")
    alpha = np.einsum("c,lch->lh", g["pair_w"], g["pairhead_w"]).astype(np.float32)
    eye = np.eye(128, dtype=np.float32)

    CB, CP, CF = 7168, 3472, 1554
    packB = np.zeros((NL, 128, CB), np.float32)
    Wqk = np.concatenate([g["qkv_w"][:, :, :C] * INV_HD,
                          g["qkv_w"][:, :, C:2 * C]], axis=2)      # [NL,256,512]
    for l in range(NL):
        for k in range(KT):
            r = slice(k * 128, (k + 1) * 128)
            packB[l, :, k * 512:(k + 1) * 512] = Wqk[l, r, :]
            packB[l, :, 1024 + k * 256:1024 + (k + 1) * 256] = \
                g["qkv_w"][l, r, 2 * C:]
            packB[l, :, 1536 + k * N1:1536 + (k + 1) * N1] = g["mlp1_w"][l, r, :]
        for n in range(NT):
            packB[l, :, 3584 + n * 256:3584 + (n + 1) * 256] = \
                g["mlp2_w"][l, n * 128:(n + 1) * 128, :]
        for hh in range(H):
            packB[l, :, 5632 + hh * 128:5632 + (hh + 1) * 128] = eye * alpha[l, hh]
            packB[l, 0:HD, 6144 + hh * 256:6144 + (hh + 1) * 256] = \
                g["out_w"][l, hh * HD:(hh + 1) * HD, :]
    packB = packB.astype(bf)

    packF = np.zeros((128, CF), np.float32)
    posb = g["pos_embed"] + g["proj_in_b"][None, :]
    for i in range(IC):
        packF[:, i * C:(i + 1) * C] = posb[i * 128:(i + 1) * 128, :]
    packF[:, 1024:1040] = g["mlp1_b"].reshape(NL, NT, 128).transpose(0, 2, 1) \
        .reshape(NL * NT, 128).T.reshape(128, NL * NT)[:, :]
    # simpler: overwrite correctly below
    b1r = g["mlp1_b"].reshape(NL, NT, 128)
    for l in range(NL):
        for n in range(NT):
            packF[:, 1024 + l * NT + n] = b1r[l, n, :]
    packF[0:IN, 1040] = g["proj_out_b"][:IN]
    packF[0:IN, 1041] = g["proj_out_b"][IN:]

    in_maps = []
    for b in range(B):
        xb = g["x"][b]
        xT = np.ascontiguousarray(xb.T)
        pinB = np.zeros((128, CP), np.float32)
        pinB[:, 0:512] = g["cond"][b].T[0:128, :]
        pinB[:, 512:1024] = g["cond"][b].T[128:256, :]
        pinB[0:IN, 1024:1536] = xT
        pinB[0:IN, 1536:2048] = -2.0 * xT
        pinB[0:1, 2048:2560] = (xb ** 2).sum(-1)[None, :]
        pinB[:, 2560:2688] = np.tril(np.ones((128, 128), np.float32))
        for k in range(KT):
            pinB[:, 2688 + k * 2 * IN:2688 + (k + 1) * 2 * IN] = \
                g["proj_out_w"][k * 128:(k + 1) * 128, :]
        pinB[0:IN, 2700:2956] = g["proj_in_w"]
        for k in range(KT):
            pinB[:, 2956 + k * C:2956 + (k + 1) * C] = \
                g["proj_cond_w"][k * 128:(k + 1) * 128, :]
        pf = packF.copy()
        pf[0:IN, 1042:1042 + L] = xT
        in_maps.append({"packB": packB, "pinB": pinB.astype(bf),
                        "packF": pf})
    return in_maps, flags


_CACHE = {}


def kernel(**inputs):
    in_maps_b, flags = host_prep(inputs)
    key = tuple(sorted(flags.items()))
    if key not in _CACHE:
        _CACHE[key] = build_program(flags)
    nc = _CACHE[key]
    in_maps = [in_maps_b[i // 4] for i in range(8)]
    res = run_bass_kernel_spmd(nc, in_maps, core_ids=list(range(8)))
    x_out = np.stack([res.results[0]["xout"].T,
                      res.results[4]["xout"].T]).astype(np.float32)
    logdet = np.array([res.results[0]["logdet"][0, 0],
                       res.results[4]["logdet"][0, 0]], np.float32)
    return x_out, logdet


# revision 35
# speedup vs baseline: 1.0083x; 1.0083x over previous
"""Trainium2 Bass kernel for nn_MetaBlock (dense transformer flow block).

Self-contained: takes FULL inputs, shards batch across 8 NeuronCores
(cores 0-3 compute batch 0, cores 4-7 batch 1 — SPMD same program),
returns full outputs (x_out [B,L,IN], logdet [B]).

Key algebraic restructurings (exact, validated vs reference):
  * The [B,L,L,C] pair tensor collapses: einsum('bijc,ch->bhij', pair, ph)
    == dist[b,i,j]*alpha[l,h] + beta[l,h] with alpha = pair_w @ ph[l,:,h];
    beta is constant per softmax row -> drops out of softmax entirely.
  * Softmax without max-subtraction (logits are O(1) by construction);
    causal mask via live-extent trimming + lower-tri multiply on the
    diagonal 128-block of e.
  * Attention fully "transposed" (logits^T[j,i]): the exp IS the PSUM
    drain, no attention-matrix transposes; alpha*dist added in-PSUM via
    a scaled-identity matmul; Z arrives free via an appended ones-column
    on V; 1/Z computed in a transposed [128, 4] layout (cheap DVE
    reciprocal) and broadcast back per-head via small DMA reshapes.
"""

import sys

sys.path.insert(0, "/opt/trn_rl_repo")

import numpy as np
import ml_dtypes

import concourse.bass as bass
import concourse.bacc as bacc
import concourse.mybir as mybir
import concourse.tile as tile
from concourse.bass_utils import run_bass_kernel_spmd
from concourse.masks import make_identity
import concourse.hw_specs as hw_specs
import concourse.bacc as _bacc_for_patch  # noqa

# Pin ACT table sets: keep set IDs stable but hide every set except the two
# we use, so the chooser never alternates between overlapping sets (each
# swap costs ~1.3us of ACT_TABLE_LOAD).
_ACT_KEEP = ("natural_log_exp_and_others", "gelu_apprx_tanh_and_others")
_orig_get_tables = hw_specs.get_activation_tables


def _pinned_tables(arch):
    t = _orig_get_tables(arch)
    return {k: (v if k in _ACT_KEEP else set()) for k, v in t.items()}


hw_specs.get_activation_tables = _pinned_tables
_bacc_for_patch.get_activation_tables = _pinned_tables

F32 = mybir.dt.float32
F32R = mybir.dt.float32r
BF16 = mybir.dt.bfloat16
AF = mybir.ActivationFunctionType
ALU = mybir.AluOpType

B, L, C, H, HD, NL, IN, EXP = 2, 512, 256, 4, 64, 2, 3, 4
N1 = EXP * C          # 1024
IC = L // 128         # 4 i-chunks
KT = C // 128         # 2 C k-tiles
NT = N1 // 128        # 8 mlp-hidden chunks
INV_HD = 1.0 / np.sqrt(HD)
EPS = 1e-5


def _r(x):
    return x.bitcast(F32R)


def build_program(flags):
    """Trace the full single-core program. flags: dict of bools enabling the
    optional bias/scale paths (emitted only when the input is nonzero)."""
    nc = bacc.Bacc("TRN2", target_bir_lowering=False, debug=False,
                   num_swdge_queues=4)

    dt_ = nc.dram_tensor
    # mega-packed inputs: per-layer bf16 weight pack, shared bf16 pack,
    # f32 pack (few large DMAs instead of ~90 small ones)
    CB = 7168   # Wqk 1024 | Wv 512 | W1 2048 | W2 2048 | aI 512 | Wo 1024(rows 0:64)
    CP = 3472   # condT 1024 | xT 512 | xT2 512 | sq 512 | tri 128 | Wout 12 | Win 256 | Wc 512 (+pad)
    CF = 1554   # posb 1024 | b1 16 | poba 1 | pobb 1 | xTf 512
    d_in = {
        "packB": dt_("packB", [NL, 128, CB], BF16, kind="ExternalInput"),
        "pinB": dt_("pinB", [128, CP], BF16, kind="ExternalInput"),
        "packF": dt_("packF", [128, CF], F32, kind="ExternalInput"),
    }
    xout_d = dt_("xout", [IN, L], F32, kind="ExternalOutput")
    ld_d = dt_("logdet", [1, 1], F32, kind="ExternalOutput")

    with tile.TileContext(nc) as tc, \
         tc.tile_pool(name="per", bufs=1) as per, \
         tc.tile_pool(name="wk", bufs=2) as wk, \
         tc.tile_pool(name="wk3", bufs=3) as wk3, \
         tc.tile_pool(name="ps", bufs=2, space="PSUM") as ps, \
         tc.tile_pool(name="psB", bufs=4, space="PSUM") as psB, \
         tc.tile_pool(name="dr", bufs=4, space="DRAM") as dr, \
         tc.tile_pool(name="wk4", bufs=4) as wk4:

        dma = nc.sync.dma_start

        # ---------------- persistent SBUF state ----------------
        # issue loads in first-use order: prologue inputs, then layer packs
        PIN_s = per.tile([128, CP], BF16, tag="PIN", name="PIN")
        # dist/h0 inputs (cols 1024:2956) first so the first matmuls start early
        dma(out=PIN_s[:, 1024:2956], in_=d_in["pinB"].ap()[:, 1024:2956])
        dma(out=PIN_s[:, 0:1024], in_=d_in["pinB"].ap()[:, 0:1024])
        dma(out=PIN_s[:, 2956:CP], in_=d_in["pinB"].ap()[:, 2956:CP])
        PF_s = per.tile([128, CF], F32, tag="PF", name="PF")
        dma(out=PF_s[:, :], in_=d_in["packF"].ap())
        PB_s = [per.tile([128, CB], BF16, tag=f"PB{l}", name=f"PB{l}")
                for l in range(NL)]
        for l in range(NL):
            dma(out=PB_s[l][:, :], in_=d_in["packB"].ap()[l])

        def pb(l, off, w):
            return PB_s[l][:, off:off + w]
        Wqk_s = [[pb(l, k * 512, 512) for k in range(KT)] for l in range(NL)]
        Wv_s = [[pb(l, 1024 + k * 256, 256) for k in range(KT)] for l in range(NL)]
        W1_s = [[pb(l, 1536 + k * N1, N1) for k in range(KT)] for l in range(NL)]
        W2_s = [[pb(l, 3584 + n * 256, 256) for n in range(NT)] for l in range(NL)]
        aI_sl = [[pb(l, 5632 + hh * 128, 128) for hh in range(H)] for l in range(NL)]
        Wo_s = [[PB_s[l][0:HD, 6144 + hh * 256:6144 + (hh + 1) * 256]
                 for hh in range(H)] for l in range(NL)]
        condT_s = [PIN_s[:, k * 512:(k + 1) * 512] for k in range(KT)]
        xT_s = PIN_s[0:IN, 1024:1024 + L]
        xT2_s = PIN_s[0:IN, 1536:1536 + L]
        sq_s = PIN_s[0:1, 2048:2048 + L]
        tri_s = PIN_s[:, 2560:2560 + 128]
        Wout_s = [PIN_s[:, 2688 + k * 2 * IN:2688 + (k + 1) * 2 * IN]
                  for k in range(KT)]
        Win_s = PIN_s[0:IN, 2700:2700 + C]
        Wc_s = [PIN_s[:, 2956 + k * C:2956 + (k + 1) * C] for k in range(KT)]
        posb_s = [PF_s[:, i * C:(i + 1) * C] for i in range(IC)]
        b1_s = [PF_s[:, 1024 + l * NT:1024 + (l + 1) * NT] for l in range(NL)]
        poba_s = PF_s[0:IN, 1040:1041]
        pobb_s = PF_s[0:IN, 1041:1042]
        xTf_s = PF_s[0:IN, 1042:1042 + L]

        opt_s = {}
        if flags["bqk"]:
            opt_s["bqk"] = [load(f"bqk{l}", [128, 2 * KT], F32,
                                 d_in["bqk"].ap()[l]) for l in range(NL)]
        for nm in ("bvb", "obb", "b2b", "ln1s", "ln1b", "ln2s", "ln2b"):
            if flags[nm]:
                opt_s[nm] = [load(f"{nm}{l}", [128, C], F32, d_in[nm].ap()[l])
                             for l in range(NL)]
        if flags["bcb"]:
            opt_s["bcb"] = load("bcb_s", [128, C], F32, d_in["bcb"].ap())

        ident = per.tile([128, 128], BF16, tag="ident", name="ident")
        make_identity(nc, ident[:, :])
        identf = per.tile([128, 128], F32, tag="identf", name="identf")
        make_identity(nc, identf[:, :])
        eps_t = per.tile([128, 1], F32, tag="eps", name="eps_t")
        nc.vector.memset(eps_t[:, :], EPS)
        ones_r = per.tile([1, L], BF16, tag="ones_r", name="ones_r")
        nc.vector.memset(ones_r[:, :], 1.0)
        ones3 = per.tile([IN, 1], F32, tag="ones3", name="ones3")
        nc.vector.memset(ones3[:, :], 1.0)

        h_s = [per.tile([128, C], F32, tag=f"h{i}", name=f"h{i}") for i in range(IC)]
        ce_s = [per.tile([128, C], F32, tag=f"ce{i}", name=f"ce{i}") for i in range(IC)]
        dist_s = [per.tile([128, L], BF16, tag=f"dist{j}", name=f"dist{j}")
                  for j in range(IC)]
        vA_s = [per.tile([128, H * (HD + 1)], BF16, tag=f"vA{j}", name=f"vA{j}")
                for j in range(IC)]
        for j in range(IC):
            for hh in range(H):
                nc.gpsimd.memset(vA_s[j][:, hh * 65 + 64: hh * 65 + 65], 1.0)


        # touch DMA-loaded operands once per engine so later consumers carry
        # at most one cross-engine wait (ISA sync-slot limit per instruction)
        tchv = per.tile([128, 1], F32, tag="tchv", name="tchv")
        tcha = per.tile([128, 1], F32, tag="tcha", name="tcha")
        nc.vector.tensor_copy(out=tchv[:, :], in_=posb_s[0][:, 0:1])
        nc.vector.tensor_copy(out=tchv[0:IN, :], in_=xTf_s[:, 0:1])
        nc.vector.tensor_copy(out=tchv[:, :].bitcast(BF16)[:, 0:1],
                              in_=tri_s[:, 0:1])
        nc.scalar.copy(tcha[:, :], b1_s[0][:, 0:1])
        nc.scalar.copy(tcha[0:IN, :], poba_s)
        nc.scalar.copy(tcha[0:IN, :], pobb_s)

        mm = nc.tensor.matmul

        def transpose128(pout, src):
            nc.tensor.transpose(pout, src, ident[:, :])

        def transpose128f(pout, src):
            nc.tensor.transpose(pout, src, identf[:, :])

        def drain(dst, src, use_act):
            if use_act:
                nc.scalar.copy(dst, src)
            else:
                nc.vector.tensor_copy(out=dst, in_=src)

        # ---------------- prologue: dist, h0, cond_emb ----------------
        for j in range(IC):
            pd = psB.tile([128, L], F32, tag="big", name=f"pd{j}")
            sl = slice(j * 128, (j + 1) * 128)
            mm(pd[:, :], xT2_s[:, sl], xT_s[:, :], start=True, stop=False)
            mm(pd[:, :], sq_s[:, sl], ones_r[:, :], start=False, stop=False)
            mm(pd[:, :], ones_r[:, sl], sq_s[:, :], start=False, stop=True)
            nc.vector.tensor_scalar_max(out=pd[:, :], in0=pd[:, :], scalar1=EPS)
            lnd = wk.tile([128, L], F32, tag="lnd", name="lnd")
            nc.scalar.activation(lnd[:, :], pd[:, :], AF.Ln,
                                 bias=eps_t[:, :], scale=1.0)
            nc.scalar.activation(dist_s[j][:, :], lnd[:, :], AF.Exp, scale=0.5)
        for i in range(IC):
            sl = slice(i * 128, (i + 1) * 128)
            ph = ps.tile([128, C], F32, tag="small", name=f"ph{i}")
            mm(ph[:, :], xT_s[:, sl], Win_s[:, :], start=True, stop=True)
            nc.vector.tensor_add(h_s[i][:, :], ph[:, :], posb_s[i][:, :])
            pc = ps.tile([128, C], F32, tag="small", name=f"pc{i}")
            for k in range(KT):
                mm(pc[:, :], condT_s[k][:, sl], Wc_s[k][:, :],
                     start=(k == 0), stop=(k == KT - 1))
            if flags["bcb"]:
                nc.vector.tensor_add(ce_s[i][:, :], pc[:, :], opt_s["bcb"][:, :])
            else:
                nc.vector.tensor_copy(out=ce_s[i][:, :], in_=pc[:, :])

        # ---------------- layers ----------------
        for l in range(NL):
            for i in range(IC):
                nc.vector.tensor_add(h_s[i][:, :], h_s[i][:, :], ce_s[i][:, :])

            def layernorm(sname, bname, out_tiles):
                for i in range(IC):
                    st = wk.tile([128, 6], F32, tag="bnst", name="st")
                    nc.vector.bn_stats(out=st[:, :], in_=h_s[i][:, :])
                    mv = wk.tile([128, 2], F32, tag="bnmv", name="mv")
                    nc.vector.bn_aggr(out=mv[:, :], in_=st[:, :])
                    sd = wk.tile([128, 1], F32, tag="sd", name="sd")
                    nc.scalar.activation(sd[:, :], mv[:, 1:2], AF.Ln,
                                         bias=eps_t[:, :], scale=1.0)
                    rs = wk.tile([128, 1], F32, tag="rs", name="rs")
                    nc.scalar.activation(rs[:, :], sd[:, :], AF.Exp, scale=-0.5)
                    at = out_tiles[i]
                    nc.vector.tensor_scalar(
                        out=at[:, :], in0=h_s[i][:, :], scalar1=mv[:, 0:1],
                        scalar2=rs[:, :], op0=ALU.subtract, op1=ALU.mult)
                    if flags[sname]:
                        nc.vector.tensor_mul(at[:, :], at[:, :], opt_s[sname][l][:, :])
                    if flags[bname]:
                        nc.vector.tensor_add(at[:, :], at[:, :], opt_s[bname][l][:, :])

            a_t = [wk3.tile([128, C], BF16, tag=f"a{i}", name=f"a{i}") for i in range(IC)]
            layernorm("ln1s", "ln1b", a_t)

            aT_t = [wk3.tile([128, L], BF16, tag=f"aT{k}", name=f"aT{k}")
                    for k in range(KT)]
            for k in range(KT):
                for i in range(IC):
                    pt = ps.tile([128, 128], BF16, tag="small", name="pt")
                    transpose128(pt[:, :], a_t[i][:, k * 128:(k + 1) * 128])
                    drain(aT_t[k][:, i * 128:(i + 1) * 128], pt[:, :], k % 2)

            qk_t = [wk3.tile([128, L], BF16, tag=f"qk{m}", name=f"qk{m}")
                    for m in range(2 * KT)]
            for m in range(2 * KT):
                pq = psB.tile([128, L], F32, tag="big", name="pq")
                for k in range(KT):
                    mm(pq[:, :], Wqk_s[l][k][:, m * 128:(m + 1) * 128],
                         aT_t[k][:, :], start=(k == 0), stop=(k == KT - 1))
                if flags["bqk"]:
                    nc.vector.tensor_scalar(
                        out=qk_t[m][:, :], in0=pq[:, :],
                        scalar1=opt_s["bqk"][l][:, m:m + 1], scalar2=None,
                        op0=ALU.add)
                else:
                    drain(qk_t[m][:, :], pq[:, :], m % 2)

            for j in range(IC):
                sl = slice(j * 128, (j + 1) * 128)
                pv = ps.tile([128, C], F32, tag="small", name="pv")
                for k in range(KT):
                    mm(pv[:, :], aT_t[k][:, sl], Wv_s[l][k][:, :],
                         start=(k == 0), stop=(k == KT - 1))
                if flags["bvb"]:
                    nc.vector.tensor_add(pv[:, :], pv[:, :], opt_s["bvb"][l][:, :])
                nc.vector.tensor_copy(
                    out=vA_s[j].rearrange("p (h x) -> p h x", h=H)[:, :, 0:HD],
                    in_=pv.rearrange("p (h x) -> p h x", h=H)[:, :, :])

            oT_t = {}
            eT_all = {}
            # phase 1: all heads' logits + exp (dense PE stream keeps HAM warm)
            for hh in range(H):
                qt = qk_t[hh // 2]
                kt_ = qk_t[KT + hh // 2]
                rq = slice((hh % 2) * HD, (hh % 2) * HD + HD)
                eT = [wk.tile([128, L], BF16, tag=f"e{hh}_{j}", name=f"e{hh}_{j}")
                      for j in range(IC)]
                for j in range(IC):
                    lo = j * 128
                    pl = psB.tile([128, L], F32, tag="big", name="pl")
                    mm(pl[:, lo:], kt_[rq, lo:lo + 128], qt[rq, lo:],
                         start=True, stop=False)
                    mm(pl[:, lo:], aI_sl[l][hh],
                       dist_s[j][:, lo:], start=False, stop=True)
                    nc.scalar.activation(eT[j][:, lo:], pl[:, lo:], AF.Exp)
                    nc.vector.tensor_mul(eT[j][:, lo:lo + 128],
                                         eT[j][:, lo:lo + 128], tri_s[:, :])
                eT_all[hh] = eT
            # phase 2: all heads' o-accumulation + 1/Z normalize
            for hh in range(H):
                eT = eT_all[hh]
                po = ps.tile([HD + 1, L], F32, tag="po", name="po")
                for j in range(IC):
                    lo = j * 128
                    mm(po[:, lo:], vA_s[j][:, hh * 65:(hh + 1) * 65],
                       eT[j][:, lo:], start=(j == 0), stop=(j == IC - 1))
                zr = wk4.tile([1, L], F32, tag="zrow", name="zr")
                nc.scalar.activation(zr[:, :], po[HD:HD + 1, :], AF.Ln)
                rr = wk4.tile([1, L], F32, tag="rrow", name="rr")
                nc.scalar.activation(rr[:, :], zr[:, :], AF.Exp, scale=-1.0)
                rb = wk4.tile([HD, L], F32, tag="rb", name="rb")
                nc.gpsimd.partition_broadcast(rb[:, :], rr[0:1, :])
                ot = wk.tile([HD, L], BF16, tag=f"oT{hh}", name=f"ot{hh}")
                nc.vector.tensor_mul(ot[:, :], po[0:HD, :], rb[:, :])
                oT_t[hh] = ot

            for i in range(IC):
                sl = slice(i * 128, (i + 1) * 128)
                pp = ps.tile([128, C], F32, tag="small", name="pp")
                for hh in range(H):
                    mm(pp[:, :], oT_t[hh][:, sl], Wo_s[l][hh][:, :],
                       start=(hh == 0), stop=(hh == H - 1))
                nc.vector.tensor_add(h_s[i][:, :], pp[:, :], h_s[i][:, :])
                if flags["obb"]:
                    nc.vector.tensor_add(h_s[i][:, :], h_s[i][:, :],
                                         opt_s["obb"][l][:, :])

            m_t = [wk3.tile([128, C], BF16, tag=f"a{i}", name=f"m{i}")
                   for i in range(IC)]
            layernorm("ln2s", "ln2b", m_t)
            mT_t = [wk3.tile([128, L], BF16, tag=f"mT{k}", name=f"mT{k}")
                    for k in range(KT)]
            for k in range(KT):
                for i in range(IC):
                    pt = ps.tile([128, 128], BF16, tag="small", name="pt")
                    transpose128(pt[:, :], m_t[i][:, k * 128:(k + 1) * 128])
                    drain(mT_t[k][:, i * 128:(i + 1) * 128], pt[:, :], k % 2)
            gT_t = [wk.tile([128, L], BF16, tag=f"gT{n}", name=f"gT{n}")
                    for n in range(NT)]
            for n in range(NT):
                pz = psB.tile([128, L], F32, tag="big", name="pz")
                for k in range(KT):
                    mm(pz[:, :], W1_s[l][k][:, n * 128:(n + 1) * 128],
                         mT_t[k][:, :], start=(k == 0), stop=(k == KT - 1))
                nc.scalar.activation(gT_t[n][:, :], pz[:, :], AF.Gelu_apprx_tanh,
                                     bias=b1_s[l][:, n:n + 1], scale=1.0)
            for i in range(IC):
                sl = slice(i * 128, (i + 1) * 128)
                pz2 = ps.tile([128, C], F32, tag="small", name="pz2")
                for n in range(NT):
                    mm(pz2[:, :], gT_t[n][:, sl], W2_s[l][n][:, :],
                       start=(n == 0), stop=(n == NT - 1))
                nc.vector.tensor_add(h_s[i][:, :], pz2[:, :], h_s[i][:, :])
                if flags["b2b"]:
                    nc.vector.tensor_add(h_s[i][:, :], h_s[i][:, :],
                                         opt_s["b2b"][l][:, :])

        # ---------------- epilogue ----------------
        hT_t = [wk3.tile([128, L], BF16, tag=f"aT{k}", name=f"hT{k}")
                for k in range(KT)]
        for k in range(KT):
            for i in range(IC):
                ptf = ps.tile([128, 128], F32, tag="small", name="ptf")
                transpose128f(ptf[:, :], h_s[i][:, k * 128:(k + 1) * 128])
                drain(hT_t[k][:, i * 128:(i + 1) * 128], ptf[:, :], k % 2)
        pya = ps.tile([IN, L], F32, tag="po", name="pya")
        pyb = ps.tile([IN, L], F32, tag="po", name="pyb")
        for k in range(KT):
            mm(pya[:, :], Wout_s[k][:, 0:IN], hT_t[k][:, :],
                 start=(k == 0), stop=(k == KT - 1))
            mm(pyb[:, :], Wout_s[k][:, IN:2 * IN], hT_t[k][:, :],
                 start=(k == 0), stop=(k == KT - 1))
        ya = wk.tile([IN, L], F32, tag="ya", name="ya")
        nc.scalar.activation(ya[:, :], pya[:, :], AF.Identity,
                             bias=poba_s, scale=1.0)
        yb = wk.tile([IN, L], F32, tag="yb", name="yb")
        nc.scalar.activation(yb[:, :], pyb[:, :], AF.Identity,
                             bias=pobb_s, scale=1.0)
        # causal shift: xa[:, i] = ya[:, i-1], col 0 = 0
        xa = wk.tile([IN, L], F32, tag="xa", name="xa")
        nc.vector.memset(xa[:, 0:1], 0.0)
        nc.vector.tensor_copy(out=xa[:, 1:L], in_=ya[:, 0:L - 1])
        xb = wk.tile([IN, L], F32, tag="xb", name="xb")
        nc.vector.memset(xb[:, 0:1], 0.0)
        nc.vector.tensor_copy(out=xb[:, 1:L], in_=yb[:, 0:L - 1])
        ea = wk.tile([IN, L], F32, tag="ea", name="ea")
        nc.scalar.activation(ea[:, :], xa[:, :], AF.Exp, scale=-1.0)
        xd = wk.tile([IN, L], F32, tag="xd", name="xd")
        nc.vector.tensor_tensor(out=xd[:, :], in0=xTf_s[:, :],
                                in1=xb[:, :], op=ALU.subtract)
        xo = wk.tile([IN, L], F32, tag="xo", name="xo")
        nc.vector.tensor_mul(xo[:, :], xd[:, :], ea[:, :])
        dma(out=xout_d.ap(), in_=xo[:, :])
        sa = wk.tile([IN, 1], F32, tag="sa", name="sa")
        nc.vector.reduce_sum(out=sa[:, :], in_=xa[:, :],
                             axis=mybir.AxisListType.X)
        pld = ps.tile([1, 1], F32, tag="po", name="pld")
        mm(pld[:, :], sa[:, :], ones3[:, :], start=True, stop=True)
        ldt = wk.tile([1, 1], F32, tag="ldt", name="ldt")
        nc.scalar.mul(ldt[:, :], pld[:, :], -1.0 / (L * IN))
        dma(out=ld_d.ap(), in_=ldt[:, :])

    nc.compile()
    return nc


def host_prep(inputs):
    """Host-side sharding/layout prep only (transposes, reshapes, tiny
    pair_w @ pairhead_w contraction, mega-packing into few DMA tensors)."""
    bf = ml_dtypes.bfloat16
    g = {k: np.asarray(v, np.float32) for k, v in inputs.items()}
    flags = {
        "bqk": bool(np.any(g["qkv_b"][:, :2 * C])),
        "bvb": bool(np.any(g["qkv_b"][:, 2 * C:])),
        "bcb": bool(np.any(g["proj_cond_b"])),
        "obb": bool(np.any(g["out_b"])),
        "b2b": bool(np.any(g["mlp2_b"])),
        "ln1s": bool(np.any(g["ln1_s"] != 1.0)),
        "ln1b": bool(np.any(g["ln1_b"])),
        "ln2s": bool(np.any(g["ln2_s"] != 1.0)),
        "ln2b": bool(np.any(g["ln2_b"])),
    }
    if any(flags.values()):
        raise NotImplementedError(f"nonzero optional bias/scale: {flags}")
    alpha = np.einsum("c,lch->lh", g["pair_w"], g["pairhead_w"]).astype(np.float32)
    eye = np.eye(128, dtype=np.float32)

    CB, CP, CF = 7168, 3472, 1554
    packB = np.zeros((NL, 128, CB), np.float32)
    Wqk = np.concatenate([g["qkv_w"][:, :, :C] * INV_HD,
                          g["qkv_w"][:, :, C:2 * C]], axis=2)      # [NL,256,512]
    for l in range(NL):
        for k in range(KT):
            r = slice(k * 128, (k + 1) * 128)
            packB[l, :, k * 512:(k + 1) * 512] = Wqk[l, r, :]
            packB[l, :, 1024 + k * 256:1024 + (k + 1) * 256] = \
                g["qkv_w"][l, r, 2 * C:]
            packB[l, :, 1536 + k * N1:1536 + (k + 1) * N1] = g["mlp1_w"][l, r, :]
        for n in range(NT):
            packB[l, :, 3584 + n * 256:3584 + (n + 1) * 256] = \
                g["mlp2_w"][l, n * 128:(n + 1) * 128, :]
        for hh in range(H):
            packB[l, :, 5632 + hh * 128:5632 + (hh + 1) * 128] = eye * alpha[l, hh]
            packB[l, 0:HD, 6144 + hh * 256:6144 + (hh + 1) * 256] = \
                g["out_w"][l, hh * HD:(hh + 1) * HD, :]
    packB = packB.astype(bf)

    packF = np.zeros((128, CF), np.float32)
    posb = g["pos_embed"] + g["proj_in_b"][None, :]
    for i in range(IC):
        packF[:, i * C:(i + 1) * C] = posb[i * 128:(i + 1) * 128, :]
    packF[:, 1024:1040] = g["mlp1_b"].reshape(NL, NT, 128).transpose(0, 2, 1) \
        .reshape(NL * NT, 128).T.reshape(128, NL * NT)[:, :]
    # simpler: overwrite correctly below
    b1r = g["mlp1_b"].reshape(NL, NT, 128)
    for l in range(NL):
        for n in range(NT):
            packF[:, 1024 + l * NT + n] = b1r[l, n, :]
    packF[0:IN, 1040] = g["proj_out_b"][:IN]
    packF[0:IN, 1041] = g["proj_out_b"][IN:]

    in_maps = []
    for b in range(B):
        xb = g["x"][b]
        xT = np.ascontiguousarray(xb.T)
        pinB = np.zeros((128, CP), np.float32)
        pinB[:, 0:512] = g["cond"][b].T[0:128, :]
        pinB[:, 512:1024] = g["cond"][b].T[128:256, :]
        pinB[0:IN, 1024:1536] = xT
        pinB[0:IN, 1536:2048] = -2.0 * xT
        pinB[0:1, 2048:2560] = (xb ** 2).sum(-1)[None, :]
        pinB[:, 2560:2688] = np.tril(np.ones((128, 128), np.float32))
        for k in range(KT):
            pinB[:, 2688 + k * 2 * IN:2688 + (k + 1) * 2 * IN] = \
                g["proj_out_w"][k * 128:(k + 1) * 128, :]
        pinB[0:IN, 2700:2956] = g["proj_in_w"]
        for k in range(KT):
            pinB[:, 2956 + k * C:2956 + (k + 1) * C] = \
                g["proj_cond_w"][k * 128:(k + 1) * 128, :]
        pf = packF.copy()
        pf[0:IN, 1042:1042 + L] = xT
        in_maps.append({"packB": packB, "pinB": pinB.astype(bf),
                        "packF": pf})
    return in_maps, flags


_CACHE = {}


def kernel(**inputs):
    in_maps_b, flags = host_prep(inputs)
    key = tuple(sorted(flags.items()))
    if key not in _CACHE:
        _CACHE[key] = build_program(flags)
    nc = _CACHE[key]
    in_maps = [in_maps_b[i // 4] for i in range(8)]
    res = run_bass_kernel_spmd(nc, in_maps, core_ids=list(range(8)))
    x_out = np.stack([res.results[0]["xout"].T,
                      res.results[4]["xout"].T]).astype(np.float32)
    logdet = np.array([res.results[0]["logdet"][0, 0],
                       res.results[4]["logdet"][0, 0]], np.float32)
    return x_out, logdet
